# revision 1
# baseline (speedup 1.0000x reference)
"""Trainium2 Bass kernel for Mobile2Former cross-attention block.

Computation (per batch b):
    xf   = x[b].reshape(C, H*W)                      # [64, 3136] keys=values
    q    = (z[b] @ Wq + bq).reshape(heads, M, C)     # [8, 6, 64]
    attn = softmax(q @ xf * C**-0.5, axis=-1)        # [8, 6, 3136]
    res  = attn @ xf.T                               # [8, 6, 64]
    out  = res.transpose(1,0,2).reshape(M, -1) @ Wo + bo + z[b]

Strategy: data-parallel over B across 8 cores (16 batches/core), batches
processed in pairs (two batches stacked on the 128 SBUF partitions, C=64
each).  QK^T is computed directly in transposed layout (attn^T[n, hm]) by
using xf chunks as the matmul stationary operand; the same stationary is
reused by an identity matmul to produce xf^T chunks for the AV matmul.
Softmax runs without max subtraction (logits are O(1)); the denominator
comes for free from a ones-column appended to the AV moving operand.
x and Wo are pre-cast to bf16 on host; the attention scale is folded into
Wq/bq on host.
"""

import sys
from contextlib import ExitStack

import numpy as np

sys.path.insert(0, "/opt/trn_rl_repo")

import concourse.bass as bass
import concourse.tile as tile
from concourse import bacc as bacc_mod
from concourse import mybir
from concourse.bass_utils import run_bass_kernel_spmd

try:
    import ml_dtypes

    BF16 = ml_dtypes.bfloat16
except ImportError:  # pragma: no cover
    import jax.numpy as jnp

    BF16 = jnp.bfloat16

N_CORES = 8
B, C, H, W = 128, 64, 56, 56
HW = H * W  # 3136
M, D = 6, 192
NH = 8
INNER = NH * C  # 512
BPC = B // N_CORES  # 16 batches per core
NPAIR = BPC // 2  # 8 pairs per core
NCHUNK = (HW + 127) // 128  # 25 (24 full + one 64-wide)

F32 = mybir.dt.float32
BF = mybir.dt.bfloat16

_CACHE = {}


def _build_nc() -> bass.Bass:
    nc = bacc_mod.Bacc()

    x_h = nc.declare_dram_parameter("x", [BPC, C, H, W], BF, isOutput=False)
    # pk1 cols: [zt0 96][zt1 96][wq0 512][wq1 512] (zt1/wq1 rows 0:64)
    pk1_h = nc.declare_dram_parameter("pk1", [128, 1216], BF, isOutput=False)
    bqt_h = nc.declare_dram_parameter("bqt", [128, 4], F32, isOutput=False)
    zbo_h = nc.declare_dram_parameter("zbo", [BPC, M, D], F32, isOutput=False)
    # pk2 cols: [ident_bf 128][wo 4*192]
    pk2_h = nc.declare_dram_parameter("pk2", [128, 896], BF, isOutput=False)
    out_h = nc.declare_dram_parameter("out", [BPC, M, D], F32, isOutput=True)

    # DRAM views
    x_r = x_h.ap().rearrange("b c h w -> (b c) (h w)")  # [1024, 3136]
    # [12(t,m), 8(pair), 192(d)]: partition q=6t+m, free (pair, d)
    zbo_r = bass.AP(
        tensor=zbo_h.ap().tensor, offset=0,
        ap=[[D, 2 * M], [2 * M * D, NPAIR], [1, D]],
    )

    with tile.TileContext(nc) as tc, ExitStack() as ctx:
        const = ctx.enter_context(tc.tile_pool(name="const", bufs=1))
        xf_pool = ctx.enter_context(tc.tile_pool(name="xf", bufs=3))
        ax_pool = ctx.enter_context(tc.tile_pool(name="ax", bufs=3))
        xts_pool = ctx.enter_context(tc.tile_pool(name="xts", bufs=3))
        small = ctx.enter_context(tc.tile_pool(name="small", bufs=3))
        at_psum = ctx.enter_context(tc.tile_pool(name="at_ps", bufs=2, space="PSUM"))
        xt_psum = ctx.enter_context(tc.tile_pool(name="xt_ps", bufs=2, space="PSUM"))
        rs_psum = ctx.enter_context(tc.tile_pool(name="rs_ps", bufs=2, space="PSUM"))
        sm_psum = ctx.enter_context(tc.tile_pool(name="sm_ps", bufs=2, space="PSUM"))

        # ---------------- phase 0: constants / projections ----------------
        # One packed critical-path load (qproj deps) on the SP HWDGE ring.
        pk1 = const.tile([128, 1216], BF)
        nc.sync.dma_start(out=pk1, in_=pk1_h.ap())
        zt0 = pk1[:, 0:96]
        zt1 = pk1[0:64, 96:192]
        wq0 = pk1[:, 192:704]
        wq1 = pk1[0:64, 704:1216]
        bqt_sb = const.tile([128, 4], F32)
        nc.sync.dma_start(out=bqt_sb, in_=bqt_h.ap())
        # Non-critical loads on the ACT HWDGE ring.
        pk2 = const.tile([128, 896], BF)
        nc.scalar.dma_start(out=pk2, in_=pk2_h.ap())
        ident_bf = pk2[:, 0:128]
        wo_sb = pk2[:, 128:896]
        zbo_sb = const.tile([12, NPAIR * D], F32)
        nc.scalar.dma_start(
            out=zbo_sb.rearrange("q (p d) -> q p d", p=NPAIR), in_=zbo_r
        )

        # Persistent xts slots (ones columns written once) and persistent
        # qT2 zero blocks (off-diagonal zeros written once).
        xts_bufs = []
        for i in range(3):
            t = const.tile([128, 5 * 132], BF, name=f"xts_buf{i}")
            tv = t.rearrange("q (j c) -> q j c", j=5)
            nc.gpsimd.memset(tv[:, :, 128:129], 1.0)
            xts_bufs.append(t)
        qT2_bufs = []
        for i in range(2):
            t = const.tile([128, 96], BF, name=f"qT2_buf{i}")
            nc.gpsimd.memset(t, 0.0)
            qT2_bufs.append(t)

        # q^T for all 16 local batches: qT_all[i, 6b+m] = (z @ Wq + bq)^T
        # chunk ii holds i in [128*ii, 128*ii+128)
        qT_all = const.tile([128, 4 * 96], BF)
        for ii in range(4):
            qp = at_psum.tile([128, 96], F32, tag="at", name=f"qp{ii}")
            nc.tensor.matmul(
                qp, lhsT=wq0[:, 128 * ii : 128 * ii + 128], rhs=zt0,
                start=True, stop=False,
            )
            nc.tensor.matmul(
                qp, lhsT=wq1[:, 128 * ii : 128 * ii + 128], rhs=zt1,
                start=False, stop=True,
            )
            nc.vector.tensor_scalar_add(
                out=qT_all[:, 96 * ii : 96 * ii + 96], in0=qp,
                scalar1=bqt_sb[:, ii : ii + 1],
            )

        # ---------------- per-pair main loop ----------------
        # column order inside a pair: hm2 = 48*b + u, u = 6*h + m.
        # Reference's q reshape is a FLAT view of [M, H*C], so the query row
        # for (h, m) is q_flat[(6h+m)//8, 64*((6h+m)%8) : +64].  With
        # u = 8*t + 2*ii + g: source chunk ii, partition half g, z-row t.
        WAVES = [(0, 5), (5, 5), (10, 5), (15, 5), (20, 5)]
        qT_all_g = qT_all.rearrange("p (hh x) -> p hh x", hh=4)  # [128, 4, 96]

        for p in range(NPAIR):
            xf = xf_pool.tile([128, HW], BF, tag="xf", name=f"xf{p}")
            nc.sync.dma_start(out=xf, in_=x_r[128 * p : 128 * (p + 1), :])

            # block-diagonal qT2: [c2, hm2]; c2 = 64*b + c
            qT2 = qT2_bufs[p % 2]
            # col = 48*b + 8*t + 2*ii + g  ->  view [q, b, ii, t, g]
            qT2_v = qT2.rearrange("q (b t ii g) -> q b ii t g", b=2, t=6, ii=4)
            for b in range(2):
                for g in range(2):
                    dst = qT2_v[64 * b : 64 * b + 64, b, :, :, g]
                    src = qT_all_g[
                        64 * g : 64 * g + 64, :, 12 * p + 6 * b : 12 * p + 6 * b + 6
                    ]
                    nc.gpsimd.tensor_copy(out=dst, in_=src)

            rsum = rs_psum.tile([96, 129], F32, tag="rs", name=f"rsum{p}")

            for w, (j0, nj) in enumerate(WAVES):
                at = at_psum.tile([128, 5 * 96], F32, tag="at", name=f"at{p}_{w}")
                xt = xt_psum.tile([128, 5 * 128], BF, tag="xt", name=f"xt{p}_{w}")
                for jj in range(nj):
                    j = j0 + jj
                    cw = 64 if j == NCHUNK - 1 else 128
                    lhs = xf[:, 128 * j : 128 * j + cw]
                    nc.tensor.matmul(
                        out=at[0:cw, 96 * jj : 96 * jj + 96], lhsT=lhs, rhs=qT2,
                        start=True, stop=True,
                    )
                    nc.tensor.transpose(
                        xt[0:cw, 128 * jj : 128 * jj + 128], lhs, ident_bf
                    )

                ax = ax_pool.tile([128, 5 * 96], BF, tag="ax", name=f"ax{p}_{w}")
                xts = xts_bufs[(p * len(WAVES) + w) % 3]
                xts_g = xts.rearrange("q (j c) -> q j c", j=5)
                xt_g = xt.rearrange("q (j c) -> q j c", j=5)
                last = w == len(WAVES) - 1

                def cp(out, in_):
                    nc.vector.tensor_copy(out=out, in_=in_)

                if not last:
                    nc.scalar.activation(
                        out=ax, in_=at, func=mybir.ActivationFunctionType.Exp
                    )
                    cp(xts_g[:, :, 0:128], xt_g)
                else:
                    nc.scalar.activation(
                        out=ax[:, 0:384], in_=at[:, 0:384],
                        func=mybir.ActivationFunctionType.Exp,
                    )
                    nc.scalar.activation(
                        out=ax[0:64, 384:480], in_=at[0:64, 384:480],
                        func=mybir.ActivationFunctionType.Exp,
                    )
                    cp(xts_g[:, 0:4, 0:128], xt_g[:, 0:4, :])
                    cp(xts_g[0:64, 4, 0:128], xt_g[0:64, 4, :])

                for jj in range(nj):
                    j = j0 + jj
                    cw = 64 if j == NCHUNK - 1 else 128
                    nc.tensor.matmul(
                        out=rsum,
                        lhsT=ax[0:cw, 96 * jj : 96 * jj + 96],
                        rhs=xts[0:cw, 132 * jj : 132 * jj + 129],
                        start=(j == 0), stop=(j == NCHUNK - 1),
                    )

            # softmax denominator is rsum[:, 128]; normalize and emit
            inv = small.tile([96, 1], F32, tag="inv", name=f"inv{p}")
            nc.vector.reciprocal(out=inv, in_=rsum[:, 128:129])
            r2n = small.tile([96, 128], BF, tag="r2n", name=f"r2n{p}")
            nc.vector.tensor_scalar_mul(out=r2n, in0=rsum[:, 0:128], scalar1=inv)

            rt = sm_psum.tile([128, 96], BF, tag="sm", name=f"rt{p}")
            nc.tensor.transpose(rt, r2n, ident_bf[0:96, 0:96])

            # fc lhsT: fcl[64*hl + c, 12*kk + 6*b + m]
            #        = rt[64*b + c, 48*b + 12*kk + 6*hl + m]   (h = 2*kk + hl)
            fcl = small.tile([128, 48], BF, tag="fcl", name=f"fcl{p}")
            fcl_g = fcl.rearrange("q (kk x) -> q kk x", kk=4)
            rt_v = rt.rearrange("q (b kk hl m) -> q b kk hl m", b=2, kk=4, hl=2)
            for hl in range(2):
                for b in range(2):
                    dst = fcl_g[64 * hl : 64 * hl + 64, :, 6 * b : 6 * b + 6]
                    src = rt_v[64 * b : 64 * b + 64, b, :, hl, :]
                    nc.vector.tensor_copy(out=dst, in_=src)

            o2 = sm_psum.tile([12, D], F32, tag="sm", name=f"o2_{p}")
            for kk in range(4):
                nc.tensor.matmul(
                    out=o2, lhsT=fcl[:, 12 * kk : 12 * kk + 12],
                    rhs=wo_sb[:, 192 * kk : 192 * kk + 192],
                    start=(kk == 0), stop=(kk == 3),
                )
            if p == 0:
                out_all = const.tile([12, NPAIR * D], F32)
            nc.vector.tensor_add(
                out=out_all[:, D * p : D * (p + 1)], in0=o2,
                in1=zbo_sb[:, D * p : D * (p + 1)],
            )
            out_rp = bass.AP(
                tensor=out_h.ap().tensor, offset=p * 2 * M * D,
                ap=[[D, 2 * M], [1, D]],
            )
            nc.scalar.dma_start(out=out_rp, in_=out_all[:, D * p : D * (p + 1)])

    return nc


def get_nc() -> bass.Bass:
    if "nc" not in _CACHE:
        nc = _build_nc()
        # The PJRT exec path serializes nc.m as-is; run Bacc's legalization
        # (wait splitting, register allocation, ...) explicitly.
        nc.finalize()
        _CACHE["nc"] = nc
    return _CACHE["nc"]


def make_in_maps(x, z, Wq, bq, Wo, bo):
    """Host-side prep + sharding into per-core input maps."""
    x = np.asarray(x, dtype=np.float32)
    z = np.asarray(z, dtype=np.float32)
    Wq = np.asarray(Wq, dtype=np.float32)
    bq = np.asarray(bq, dtype=np.float32)
    Wo = np.asarray(Wo, dtype=np.float32)
    bo = np.asarray(bo, dtype=np.float32)

    scale = np.float32(C ** -0.5)
    x_bf = x.astype(BF16)
    wq_s = (Wq * scale).astype(BF16)
    bqt = (bq * scale).reshape(4, 128).T  # [128, 4], chunk ii in col ii
    wo_bf = Wo.astype(BF16)
    zbo = (z + bo[None, None, :]).astype(np.float32)
    # pk2 = [ident 128 | wo 4*192] with wo[p, 192k+d] = Wo[128k+p, d]
    pk2 = np.zeros((128, 896), dtype=BF16)
    pk2[:, 0:128] = np.eye(128, dtype=BF16)
    pk2[:, 128:896] = np.ascontiguousarray(
        wo_bf.reshape(4, 128, D).transpose(1, 0, 2).reshape(128, 4 * D)
    )

    in_maps = []
    for i in range(N_CORES):
        s = slice(i * BPC, (i + 1) * BPC)
        # zt[d, 6*b_local + m] = z[core_base + b_local, m, d]
        zt = z[s].reshape(BPC * M, D).T.astype(BF16)
        pk1 = np.zeros((128, 1216), dtype=BF16)
        pk1[:, 0:96] = zt[0:128]
        pk1[0:64, 96:192] = zt[128:192]
        pk1[:, 192:704] = wq_s[0:128]
        pk1[0:64, 704:1216] = wq_s[128:192]
        in_maps.append(
            {
                "x": x_bf[s],
                "pk1": pk1,
                "bqt": np.ascontiguousarray(bqt),
                "zbo": zbo[s],
                "pk2": pk2,
            }
        )
    return in_maps


def kernel(**inputs) -> np.ndarray:
    nc = get_nc()
    in_maps = make_in_maps(
        inputs["x"], inputs["z"], inputs["Wq"], inputs["bq"],
        inputs["Wo"], inputs["bo"],
    )
    res = run_bass_kernel_spmd(nc, in_maps, list(range(N_CORES)))
    out = np.concatenate(
        [np.asarray(res.results[i]["out"]) for i in range(N_CORES)], axis=0
    )
    return out.astype(np.float32)



# revision 6
# speedup vs baseline: 1.0370x; 1.0370x over previous
"""Trainium2 Bass kernel for Mobile2Former cross-attention block.

Computation (per batch b):
    xf   = x[b].reshape(C, H*W)                      # [64, 3136] keys=values
    q    = (z[b] @ Wq + bq).reshape(heads, M, C)     # [8, 6, 64]
    attn = softmax(q @ xf * C**-0.5, axis=-1)        # [8, 6, 3136]
    res  = attn @ xf.T                               # [8, 6, 64]
    out  = res.transpose(1,0,2).reshape(M, -1) @ Wo + bo + z[b]

Strategy: data-parallel over B across 8 cores (16 batches/core).  Batches are
processed in QUADS (4 batches) using fp8e4 DoubleRow matmuls: the contraction
dim packs two 128-row slabs (two batch-PAIRS block-diagonal for QK; two
consecutive 128-key chunks for AV), giving 2x tensor-engine throughput.  The
AV operand x^T arrives pre-transposed from the host (kills all on-chip PE
transposes), with a ones-column per slab yielding the softmax denominator for
free.  Softmax runs without max subtraction (logits are O(1)); normalization
happens once on the [96,128] result.  The Scalar engine runs ONLY the exp
waves (its 16us of exp work is the critical resource); all DMA issue is on
sync/vector, copies on gpsimd/vector.  The attention scale is folded into
Wq/bq on host; output Wo projection is batched over all 8 pairs at the end.
"""

import sys
from contextlib import ExitStack

import numpy as np

sys.path.insert(0, "/opt/trn_rl_repo")

import concourse.bass as bass
import concourse.tile as tile
from concourse import bacc as bacc_mod
from concourse import mybir
from concourse.bass_utils import run_bass_kernel_spmd

import ml_dtypes

BF16 = ml_dtypes.bfloat16
FP8 = ml_dtypes.float8_e4m3

N_CORES = 8
B, C, H, W = 128, 64, 56, 56
HW = H * W  # 3136
M, D = 6, 192
NH = 8
INNER = NH * C  # 512
BPC = B // N_CORES  # 16 batches per core
NQUAD = BPC // 4  # 4
NPAIR = BPC // 2  # 8
NCHUNK = (HW + 127) // 128  # 25 (24 full + one 64-wide)
NDC = (NCHUNK + 1) // 2  # 13 double-chunks (last has a dead slab)
PVW = 132  # AV moving cols per slab: 128 c + 1 ones + 3 pad

F32 = mybir.dt.float32
BF = mybir.dt.bfloat16
F8 = mybir.dt.float8e4
DR = mybir.MatmulPerfMode.DoubleRow

_CACHE = {}


def _build_nc() -> bass.Bass:
    nc = bacc_mod.Bacc()

    # QK stationary: [quad*128 part (2b x 64c), 25 j, 2 slab(pair), 128 n] fp8
    px_h = nc.declare_dram_parameter("px", [NQUAD * 128, NCHUNK * 2 * 128], F8,
                                     isOutput=False)
    # AV moving: [pair*128 part (n%128), 13 dc, 2 slab(n chunk), 132] fp8
    pv_h = nc.declare_dram_parameter("pv", [NPAIR * 128, NDC * 2 * PVW], F8,
                                     isOutput=False)
    # pk1 cols: [zt0 96][zt1 96][wq0 512][wq1 512] (zt1/wq1 rows 0:64)
    pk1_h = nc.declare_dram_parameter("pk1", [128, 1216], BF, isOutput=False)
    bqt_h = nc.declare_dram_parameter("bqt", [128, 4], F32, isOutput=False)
    # pk2 cols: [ident_bf 128][wo 4*192]
    pk2_h = nc.declare_dram_parameter("pk2", [128, 896], BF, isOutput=False)
    # z + bo in output layout: row 12r + 6bb + m
    zz_h = nc.declare_dram_parameter("zz", [96, D], F32, isOutput=False)
    out_h = nc.declare_dram_parameter("out", [96, D], F32, isOutput=True)

    with tile.TileContext(nc) as tc, ExitStack() as ctx:
        const = ctx.enter_context(tc.tile_pool(name="const", bufs=1))
        px_pool = ctx.enter_context(tc.tile_pool(name="px", bufs=2))
        pv_pool = ctx.enter_context(tc.tile_pool(name="pv", bufs=3))
        small = ctx.enter_context(tc.tile_pool(name="small", bufs=3))
        at_ps = ctx.enter_context(tc.tile_pool(name="at_ps", bufs=2, space="PSUM"))
        rs_ps = ctx.enter_context(tc.tile_pool(name="rs_ps", bufs=2, space="PSUM"))

        # ---------------- phase 0: warmup + constants ----------------
        # Pre-warm the Exp activation table while DMAs run.
        warm = const.tile([128, 8], F32)
        nc.vector.memset(warm, 0.0)
        warm2 = const.tile([128, 8], F32)
        nc.scalar.activation(out=warm2, in_=warm,
                             func=mybir.ActivationFunctionType.Exp)

        pk1 = const.tile([128, 1216], BF)
        nc.sync.dma_start(out=pk1, in_=pk1_h.ap())
        zt0 = pk1[:, 0:96]
        zt1 = pk1[0:64, 96:192]
        wq0 = pk1[:, 192:704]
        wq1 = pk1[0:64, 704:1216]
        bqt_sb = const.tile([128, 4], F32)
        nc.sync.dma_start(out=bqt_sb, in_=bqt_h.ap())
        pk2 = const.tile([128, 896], BF)
        nc.sync.dma_start(out=pk2, in_=pk2_h.ap())
        ident_bf = pk2[:, 0:128]
        wo_sb = pk2[:, 128:896]
        zz_sb = const.tile([96, D], F32)
        nc.sync.dma_start(out=zz_sb, in_=zz_h.ap())

        # ---------------- qproj: q^T for all 16 local batches ----------------
        # qT_all[p, ii, 6*b_local + m] = (z @ Wq + bq)^T fp8, inner = 128*ii + p
        qT_all = const.tile([128, 4 * 96], F8)
        qT_all_g = qT_all.rearrange("p (ii x) -> p ii x", ii=4)
        qp = at_ps.tile([128, 4 * 96], F32, tag="at", name="qp")
        qp_g = qp.rearrange("p (ii x) -> p ii x", ii=4)
        for ii in range(4):
            nc.tensor.matmul(
                qp_g[:, ii, :], lhsT=wq0[:, 128 * ii: 128 * ii + 128], rhs=zt0,
                start=True, stop=False,
            )
            nc.tensor.matmul(
                qp_g[:, ii, :], lhsT=wq1[:, 128 * ii: 128 * ii + 128], rhs=zt1,
                start=False, stop=True,
            )
            nc.vector.tensor_scalar_add(
                out=qT_all_g[:, ii, :], in0=qp_g[:, ii, :],
                scalar1=bqt_sb[:, ii: ii + 1],
            )

        # qT4[g]: QK moving operand per quad, [128 (c2), 2 slab(pair), 192] fp8
        # valid block: slab i, cols 96i:96i+96 holds qT2 of pair 2g+i; rest 0.
        qT4 = []
        for g in range(NQUAD):
            t = const.tile([128, 2 * 192], F8, name=f"qT4_{g}")
            nc.gpsimd.memset(t, 0.0)
            qT4.append(t)
        for g in range(NQUAD):
            tv = qT4[g].rearrange("q (i c) -> q i c", i=2)
            for i in range(2):
                p = 2 * g + i  # pair index
                # col within block = 48*bb + 8*t + 2*ii + gh
                blk = tv[:, i, 96 * i: 96 * i + 96].rearrange(
                    "q (bb t ii gh) -> q bb ii t gh", bb=2, t=6, ii=4
                )
                for bb in range(2):
                    for gh in range(2):
                        dst = blk[64 * bb: 64 * bb + 64, bb, :, :, gh]
                        src = qT_all_g[
                            64 * gh: 64 * gh + 64, :, 12 * p + 6 * bb: 12 * p + 6 * bb + 6
                        ]
                        eng = nc.gpsimd if (g % 2 == 0) else nc.vector
                        eng.tensor_copy(out=dst, in_=src)

        # ax buffers: exp output / AV stationary, [128, 13 dc, 2 slab, 192] fp8.
        # Dead tail regions pre-zeroed once (exp never writes them).
        ax_bufs = []
        for i in range(2):
            t = const.tile([128, NDC * 2 * 192], F8, name=f"ax_buf{i}")
            tv = t.rearrange("p (d i c) -> p d i c", d=NDC, i=2)
            nc.gpsimd.memset(tv[64:128, NDC - 1, 0, :], 0.0)
            nc.gpsimd.memset(tv[:, NDC - 1, 1, :], 0.0)
            ax_bufs.append(t)

        # fcl_all: Wo-projection stationary for all 8 pairs,
        # fcl_all[64*hl + c, kk, 12*r + 6*bb + m] bf16
        fcl_all = const.tile([128, 4 * 96], BF)
        fcl_g = fcl_all.rearrange("q (kk x) -> q kk x", kk=4)

        # ---------------- main loop: per quad ----------------
        WAVES = [(0, 6), (6, 6), (12, 6), (18, 6)]  # + tail chunk 24

        for g in range(NQUAD):
            pxg = px_pool.tile([128, NCHUNK * 2 * 128], F8, tag="px", name=f"px{g}")
            nc.sync.dma_start(out=pxg, in_=px_h.ap()[128 * g: 128 * (g + 1), :])
            pxv = pxg.rearrange("p (j i t) -> p j i t", j=NCHUNK, i=2)
            pvt = []
            for i in range(2):
                t = pv_pool.tile([128, NDC * 2 * PVW], F8, tag="pv",
                                 name=f"pv{2 * g + i}")
                r = 2 * g + i
                nc.sync.dma_start(out=t, in_=pv_h.ap()[128 * r: 128 * (r + 1), :])
                pvt.append(t.rearrange("p (d i c) -> p d i c", d=NDC, i=2))

            ax = ax_bufs[g % 2]
            axv = ax.rearrange("p (d i c) -> p d i c", d=NDC, i=2)
            qT4v = qT4[g].rearrange("q (i c) -> q i c", i=2)

            # both pair accumulators share one PSUM bank (disjoint columns)
            rsb = rs_ps.tile([96, 2 * PVW], F32, tag="rs", name=f"rsum{g}")
            rsum = [rsb[:, 0:PVW], rsb[:, PVW: 2 * PVW]]

            for w, (j0, nj) in enumerate(WAVES):
                at = at_ps.tile([128, 6 * 256], F32, tag="at", name=f"at{g}_{w}")
                atv = at.rearrange("p (j c) -> p j c", j=6)
                for jj in range(nj):
                    j = j0 + jj
                    nc.tensor.matmul(
                        atv[:, jj, 0:192], lhsT=pxv[:, j, :, :], rhs=qT4v,
                        perf_mode=DR, start=True, stop=True,
                    )
                nc.scalar.activation(
                    out=axv[:, 3 * w: 3 * w + 3, :, :], in_=atv[:, :, 0:192],
                    func=mybir.ActivationFunctionType.Exp,
                )
                for d in range(3 * w, 3 * w + 3):
                    for i in range(2):
                        nc.tensor.matmul(
                            rsum[i], lhsT=axv[:, d, :, 96 * i: 96 * i + 96],
                            rhs=pvt[i][:, d, :, :],
                            perf_mode=DR, start=(d == 0), stop=False,
                        )

            # tail chunk 24 (64 keys): slab1 of dc 12 is pre-zeroed
            at_t = at_ps.tile([64, 192], F32, tag="at", name=f"att{g}")
            nc.tensor.matmul(
                at_t, lhsT=pxv[:, NCHUNK - 1, :, 0:64], rhs=qT4v,
                perf_mode=DR, start=True, stop=True,
            )
            nc.scalar.activation(
                out=axv[0:64, NDC - 1, 0, :], in_=at_t,
                func=mybir.ActivationFunctionType.Exp,
            )
            d = NDC - 1
            for i in range(2):
                nc.tensor.matmul(
                    rsum[i], lhsT=axv[:, d, :, 96 * i: 96 * i + 96],
                    rhs=pvt[i][:, d, :, :],
                    perf_mode=DR, start=False, stop=True,
                )

            # normalize + transpose + fcl assembly per pair
            for i in range(2):
                r = 2 * g + i
                inv = small.tile([96, 1], F32, tag="inv", name=f"inv{r}")
                nc.vector.reciprocal(out=inv, in_=rsum[i][:, 128:129])
                r2n = small.tile([96, 128], BF, tag="r2n", name=f"r2n{r}")
                nc.vector.tensor_scalar_mul(out=r2n, in0=rsum[i][:, 0:128],
                                            scalar1=inv)
                rt = rs_ps.tile([128, 96], BF, tag="rs", name=f"rt{r}")
                nc.tensor.transpose(rt, r2n, ident_bf[0:96, 0:96])
                rt_v = rt.rearrange("q (bb kk hl m) -> q bb kk hl m",
                                    bb=2, kk=4, hl=2)
                for hl in range(2):
                    for bb in range(2):
                        dst = fcl_g[64 * hl: 64 * hl + 64, :,
                                    12 * r + 6 * bb: 12 * r + 6 * bb + 6]
                        src = rt_v[64 * bb: 64 * bb + 64, bb, :, hl, :]
                        nc.vector.tensor_copy(out=dst, in_=src)

        # ---------------- tail: batched Wo projection ----------------
        o2 = rs_ps.tile([96, D], F32, tag="rs", name="o2")
        for kk in range(4):
            nc.tensor.matmul(
                out=o2, lhsT=fcl_g[:, kk, :],
                rhs=wo_sb[:, 192 * kk: 192 * kk + 192],
                start=(kk == 0), stop=(kk == 3),
            )
        out_sb = const.tile([96, D], F32)
        nc.vector.tensor_add(out=out_sb, in0=o2, in1=zz_sb)
        nc.sync.dma_start(out=out_h.ap(), in_=out_sb)

    return nc


def get_nc() -> bass.Bass:
    if "nc" not in _CACHE:
        nc = _build_nc()
        # The PJRT exec path serializes nc.m as-is; run Bacc's legalization
        # (wait splitting, register allocation, ...) explicitly.
        nc.finalize()
        _CACHE["nc"] = nc
    return _CACHE["nc"]


def make_in_maps(x, z, Wq, bq, Wo, bo):
    """Host-side prep + sharding into per-core input maps."""
    x = np.asarray(x, dtype=np.float32)
    z = np.asarray(z, dtype=np.float32)
    Wq = np.asarray(Wq, dtype=np.float32)
    bq = np.asarray(bq, dtype=np.float32)
    Wo = np.asarray(Wo, dtype=np.float32)
    bo = np.asarray(bo, dtype=np.float32)

    scale = np.float32(C ** -0.5)
    x_f8 = x.reshape(B, C, HW).astype(FP8)
    wq_s = (Wq * scale).astype(BF16)
    bqt = (bq * scale).reshape(4, 128).T  # [128, 4], chunk ii in col ii
    wo_bf = Wo.astype(BF16)
    # pk2 = [ident 128 | wo 4*192] with wo[p, 192k+d] = Wo[128k+p, d]
    pk2 = np.zeros((128, 896), dtype=BF16)
    pk2[:, 0:128] = np.eye(128, dtype=BF16)
    pk2[:, 128:896] = np.ascontiguousarray(
        wo_bf.reshape(4, 128, D).transpose(1, 0, 2).reshape(128, 4 * D)
    )

    in_maps = []
    for ci in range(N_CORES):
        s = slice(ci * BPC, (ci + 1) * BPC)
        xc = x_f8[s]  # [16, 64, 3136]

        # px: QK stationary. [g, 128 (bb*64+c), j, i, t] with
        # px[g, 64bb+c, j, i, t] = x[4g + 2i + bb, c, 128j + t]
        xp = np.zeros((BPC, C, NCHUNK, 128), dtype=FP8)
        xp[:, :, :24, :] = xc[:, :, : 24 * 128].reshape(BPC, C, 24, 128)
        xp[:, :, 24, :64] = xc[:, :, 24 * 128:]
        xq = xp.reshape(NQUAD, 2, 2, C, NCHUNK, 128)  # [g, i, bb, c, j, t]
        px = np.ascontiguousarray(xq.transpose(0, 2, 3, 4, 1, 5)).reshape(
            NQUAD * 128, NCHUNK * 2 * 128
        )

        # pv: AV moving (x^T with ones col). [r, 128 t, d, i, cc]
        # pv[r, t, d, i, cc] = x[2r + cc//64, cc%64, 256d + 128i + t]
        xt_pad = np.zeros((NPAIR, NDC * 256, PVW), dtype=FP8)
        xt_pad[:, :HW, :128] = (
            xc.reshape(NPAIR, 2, C, HW).transpose(0, 3, 1, 2).reshape(NPAIR, HW, 128)
        )
        xt_pad[:, :HW, 128] = np.float32(1.0)
        pv = np.ascontiguousarray(
            xt_pad.reshape(NPAIR, NDC, 2, 128, PVW).transpose(0, 3, 1, 2, 4)
        ).reshape(NPAIR * 128, NDC * 2 * PVW)

        # zt[d, 6*b_local + m] = z[core_base + b_local, m, d]
        zt = z[s].reshape(BPC * M, D).T.astype(BF16)
        pk1 = np.zeros((128, 1216), dtype=BF16)
        pk1[:, 0:96] = zt[0:128]
        pk1[0:64, 96:192] = zt[128:192]
        pk1[:, 192:704] = wq_s[0:128]
        pk1[0:64, 704:1216] = wq_s[128:192]

        # zz[12r + 6bb + m] = z[2r + bb, m] + bo
        zz = (z[s] + bo[None, None, :]).reshape(96, D).astype(np.float32)

        in_maps.append(
            {
                "px": px,
                "pv": pv,
                "pk1": pk1,
                "bqt": np.ascontiguousarray(bqt),
                "pk2": pk2,
                "zz": zz,
            }
        )
    return in_maps


def kernel(**inputs) -> np.ndarray:
    nc = get_nc()
    in_maps = make_in_maps(
        inputs["x"], inputs["z"], inputs["Wq"], inputs["bq"],
        inputs["Wo"], inputs["bo"],
    )
    res = run_bass_kernel_spmd(nc, in_maps, list(range(N_CORES)))
    out = np.concatenate(
        [
            np.asarray(res.results[i]["out"]).reshape(BPC, M, D)
            for i in range(N_CORES)
        ],
        axis=0,
    )
    return out.astype(np.float32)


# revision 10
# speedup vs baseline: 1.2582x; 1.2133x over previous
"""Trainium2 Bass kernel for Mobile2Former cross-attention block.

Computation (per batch b):
    xf   = x[b].reshape(C, H*W)                      # [64, 3136] keys=values
    q    = (z[b] @ Wq + bq).reshape(heads, M, C)     # [8, 6, 64]
    attn = softmax(q @ xf * C**-0.5, axis=-1)        # [8, 6, 3136]
    res  = attn @ xf.T                               # [8, 6, 64]
    out  = res.transpose(1,0,2).reshape(M, -1) @ Wo + bo + z[b]

Strategy: data-parallel over B across 8 cores (16 batches/core).  Batches are
processed in QUADS (4 batches) using fp8e4 DoubleRow matmuls: the contraction
dim packs two 128-row slabs (two batch-PAIRS block-diagonal for QK; two
consecutive 128-key chunks for AV), giving 2x tensor-engine throughput.  The
AV operand x^T arrives pre-transposed from the host (no on-chip transposes),
with a ones-column per slab yielding the softmax denominator for free.
Softmax runs without max subtraction (logits are O(1)).

The Scalar engine runs ONLY the exp waves (its ~20us of exp work is the
critical resource): per quad, 5 waves of 5 key-chunks land densely packed in
PSUM ([2 banks, 480 used of 512]) so each exp is one [128,2,480] read.  The
per-quad PE program issues all QK waves first, then the AV accumulations, and
the exp-w4-dependent AV tail + normalize/transpose of quad g runs after QK of
quad g+1, so exp never waits on PE.  The q projection folds bias via a
ones-row in the stationary and lands in the block-diagonal fp8 layout with 4
strided DVE conversion copies.  All DMA issue is on sync; Wo projection is
batched over all 8 pairs at the end.
"""

import sys
from contextlib import ExitStack

import numpy as np

sys.path.insert(0, "/opt/trn_rl_repo")

import concourse.bass as bass
import concourse.tile as tile
from concourse import bacc as bacc_mod
from concourse import mybir
from concourse.bass_utils import run_bass_kernel_spmd

import ml_dtypes

BF16 = ml_dtypes.bfloat16
FP8 = ml_dtypes.float8_e4m3

N_CORES = 8
B, C, H, W = 128, 64, 56, 56
HW = H * W  # 3136
M, D = 6, 192
NH = 8
INNER = NH * C  # 512
BPC = B // N_CORES  # 16 batches per core
NQUAD = BPC // 4  # 4
NPAIR = BPC // 2  # 8
NCHUNK = (HW + 127) // 128  # 25 (24 full + one 64-wide)
NDC = (NCHUNK + 1) // 2  # 13 double-chunks (last has a dead slab)
PVW = 132  # AV moving cols per slab: 128 c + 1 ones + 3 pad

F32 = mybir.dt.float32
BF = mybir.dt.bfloat16
F8 = mybir.dt.float8e4
DR = mybir.MatmulPerfMode.DoubleRow

# per-wave chunk sub-offsets in the at tile (f32 elems); chunk jj=2 is split
# 96/96 across the bank boundary
AT_OFF = [(0, 192), (192, 192), (384, 96), (512, 96), (608, 192), (800, 192)]

_CACHE = {}


def _build_nc() -> bass.Bass:
    nc = bacc_mod.Bacc()

    # QK stationary: [quad*128 part (2b x 64c), 25 j, 2 slab(pair), 128 n] fp8
    px_h = nc.declare_dram_parameter("px", [NQUAD * 128, NCHUNK * 2 * 128], F8,
                                     isOutput=False)
    # AV moving: [pair*128 part (n%128), 13 dc, 2 slab(n chunk), 132] fp8
    pv_h = nc.declare_dram_parameter("pv", [NPAIR * 128, NDC * 2 * PVW], F8,
                                     isOutput=False)
    # pk1 cols: [zt0 96][zt1+ones 96][wq0 512][wq1+bq 512] (rows 0:65 for 1-blks)
    pk1_h = nc.declare_dram_parameter("pk1", [128, 1216], BF, isOutput=False)
    # pk2 cols: [ident_bf 128][wo 4*192]
    pk2_h = nc.declare_dram_parameter("pk2", [128, 896], BF, isOutput=False)
    # z + bo in output layout: row 12r + 6bb + m
    zz_h = nc.declare_dram_parameter("zz", [96, D], F32, isOutput=False)
    out_h = nc.declare_dram_parameter("out", [96, D], F32, isOutput=True)

    with tile.TileContext(nc) as tc, ExitStack() as ctx:
        const = ctx.enter_context(tc.tile_pool(name="const", bufs=1))
        px_pool = ctx.enter_context(tc.tile_pool(name="px", bufs=3))
        pv_pool = ctx.enter_context(tc.tile_pool(name="pv", bufs=4))
        small = ctx.enter_context(tc.tile_pool(name="small", bufs=3))
        at_ps = ctx.enter_context(tc.tile_pool(name="at_ps", bufs=3, space="PSUM"))
        rs_ps = ctx.enter_context(tc.tile_pool(name="rs_ps", bufs=2, space="PSUM"))

        # ---------------- phase 0: warmup + constants ----------------
        # Pre-warm the Exp activation table while DMAs run.
        warm = const.tile([128, 8], F32)
        nc.vector.memset(warm, 0.0)
        warm2 = const.tile([128, 8], F32)
        nc.scalar.activation(out=warm2, in_=warm,
                             func=mybir.ActivationFunctionType.Exp)

        pk1 = const.tile([128, 1216], BF)
        nc.sync.dma_start(out=pk1, in_=pk1_h.ap())
        zt0 = pk1[:, 0:96]
        zt1 = pk1[0:65, 96:192]
        wq0 = pk1[:, 192:704]
        wq1 = pk1[0:65, 704:1216]

        # first quad's x early; remaining loads interleave below
        px_t = []
        for g in range(NQUAD):
            px_t.append(px_pool.tile([128, NCHUNK * 2 * 128], F8, tag="px",
                                     name=f"px{g}"))
        pv_t = []
        for r in range(NPAIR):
            pv_t.append(pv_pool.tile([128, NDC * 2 * PVW], F8, tag="pv",
                                     name=f"pv{r}"))

        def load_px(g):
            nc.sync.dma_start(out=px_t[g], in_=px_h.ap()[128 * g: 128 * (g + 1), :])

        def load_pv(r):
            nc.sync.dma_start(out=pv_t[r], in_=pv_h.ap()[128 * r: 128 * (r + 1), :])

        load_px(0)
        pk2 = const.tile([128, 896], BF)
        nc.sync.dma_start(out=pk2, in_=pk2_h.ap())
        ident_bf = pk2[:, 0:128]
        wo_sb = pk2[:, 128:896]
        zz_sb = const.tile([96, D], F32)
        nc.sync.dma_start(out=zz_sb, in_=zz_h.ap())
        load_pv(0)
        load_pv(1)
        load_px(1)

        # ---------------- qproj ----------------
        # qp[64gh+c, ii, 6*bl + t] = q^T[128ii + 64gh + c, batch bl, row t]
        # (bias folded: zt1 row 64 = ones, wq1 row 64 = bq*scale)
        qp = at_ps.tile([128, 4 * 96], F32, tag="at", name="qp")
        qp_g = qp.rearrange("p (ii x) -> p ii x", ii=4)
        for ii in range(4):
            nc.tensor.matmul(
                qp_g[:, ii, :], lhsT=wq0[:, 128 * ii: 128 * ii + 128], rhs=zt0,
                start=True, stop=False,
            )
            nc.tensor.matmul(
                qp_g[:, ii, :], lhsT=wq1[:, 128 * ii: 128 * ii + 128], rhs=zt1,
                start=False, stop=True,
            )

        # qT4big: QK moving operand, [128 (c2), 4 g, 2 slab(pair), 192] fp8;
        # valid block of quad g, slab i: cols 96i:96i+96 (block-diagonal).
        # In-block col = 48bb + 8t + 2ii + gh; flat offset within a g-block is
        # 288i + 48bb + gh + 8t + 2ii.
        qT4big = const.tile([128, NQUAD * 2 * 192], F8)
        nc.gpsimd.memset(qT4big, 0.0)
        qT4_gv = qT4big.rearrange("p (g y) -> p g y", g=NQUAD)
        # src: qp free = (ii g i b2 t) with strides (96, 24, 12, 6, 1)
        qp_r = qp.rearrange("p (ii g i b2 t) -> p g t ii i b2",
                            ii=4, g=NQUAD, i=2, b2=2)
        for i in range(2):
            for bb in range(2):
                for gh in range(2):
                    base = 288 * i + 48 * bb
                    dst = qT4_gv[64 * bb: 64 * bb + 64, :, base: base + 48]
                    dst = dst.rearrange("p g (t ii w) -> p g t ii w", t=6, ii=4)
                    dst = dst[:, :, :, :, gh]
                    src = qp_r[64 * gh: 64 * gh + 64, :, :, :, i, bb]
                    if i == 0:
                        nc.vector.tensor_copy(out=dst, in_=src)
                    else:
                        nc.scalar.activation(
                            out=dst, in_=src,
                            func=mybir.ActivationFunctionType.Copy,
                        )
        qT4v_all = qT4big.rearrange("p (g i c) -> p g i c", g=NQUAD, i=2)

        # ax buffers: exp output / AV stationary, [128, 13 dc, 2 slab, 192] fp8.
        # Dead tail region (dc12 slab1) pre-zeroed once; exp never writes it.
        ax_bufs = []
        for i in range(2):
            t = const.tile([128, NDC * 2 * 192], F8, name=f"ax_buf{i}")
            tv = t.rearrange("p (d i c) -> p d i c", d=NDC, i=2)
            nc.gpsimd.memset(tv[64:128, NDC - 1, 0, :], 0.0)
            nc.gpsimd.memset(tv[:, NDC - 1, 1, :], 0.0)
            ax_bufs.append(t)

        # fcl_all: Wo-projection stationary for all 8 pairs,
        # fcl_all[64*hl + c, kk, 12*r + 6*bb + m] bf16
        fcl_all = const.tile([128, 4 * 96], BF)
        fcl_g = fcl_all.rearrange("q (kk x) -> q kk x", kk=4)

        # ---------------- main loop: per quad ----------------
        # deferred state from quad g-1 for the cross-quad AV tail
        pend = {}

        def do_qk(g):
            pxv = px_t[g].rearrange("p (j i t) -> p j i t", j=NCHUNK, i=2)
            qT4v = qT4v_all[:, g, :, :]
            ats = []
            for w in range(5):
                at = at_ps.tile([128, 1024], F32, tag="at", name=f"at{g}_{w}")
                for jj in range(5):
                    j = 5 * w + jj
                    cw = 64 if j == NCHUNK - 1 else 128
                    if jj == 2:
                        nc.tensor.matmul(
                            at[0:cw, 384:480], lhsT=pxv[:, j, :, 0:cw],
                            rhs=qT4v[:, :, 0:96], perf_mode=DR,
                            start=True, stop=True,
                        )
                        nc.tensor.matmul(
                            at[0:cw, 512:608], lhsT=pxv[:, j, :, 0:cw],
                            rhs=qT4v[:, :, 96:192], perf_mode=DR,
                            start=True, stop=True,
                        )
                    else:
                        o = AT_OFF[jj if jj < 2 else jj + 1][0]
                        nc.tensor.matmul(
                            at[0:cw, o: o + 192], lhsT=pxv[:, j, :, 0:cw],
                            rhs=qT4v, perf_mode=DR, start=True, stop=True,
                        )
                ats.append(at)
            return ats

        def do_exp(g, ats, axf):
            for w in range(5):
                in_v = ats[w].rearrange("p (b c) -> p b c", b=2)[:, :, 0:480]
                nc.scalar.activation(
                    out=axf[:, 960 * w: 960 * (w + 1)], in_=in_v,
                    func=mybir.ActivationFunctionType.Exp,
                )

        def do_av(g, axv, rsum, d0, d1):
            for d in range(d0, d1):
                for i in range(2):
                    nc.tensor.matmul(
                        rsum[i], lhsT=axv[:, d, :, 96 * i: 96 * i + 96],
                        rhs=pv_t[2 * g + i].rearrange(
                            "p (d i c) -> p d i c", d=NDC, i=2)[:, d, :, :],
                        perf_mode=DR, start=(d == 0), stop=(d == NDC - 1),
                    )

        def finish_quad(p):
            # AV tail (needs last exp of quad p['g']), then normalize+fcl
            g = p["g"]
            do_av(g, p["axv"], p["rsum"], 10, NDC)
            for i in range(2):
                r = 2 * g + i
                inv = small.tile([96, 1], F32, tag="inv", name=f"inv{r}")
                nc.vector.reciprocal(out=inv, in_=p["rsum"][i][:, 128:129])
                r2n = small.tile([96, 128], BF, tag="r2n", name=f"r2n{r}")
                nc.vector.tensor_scalar_mul(out=r2n, in0=p["rsum"][i][:, 0:128],
                                            scalar1=inv)
                rt = rs_ps.tile([128, 96], BF, tag="rs", name=f"rt{r}")
                nc.tensor.transpose(rt, r2n, ident_bf[0:96, 0:96])
                rt_v = rt.rearrange("q (bb kk hl m) -> q bb kk hl m",
                                    bb=2, kk=4, hl=2)
                for hl in range(2):
                    for bb in range(2):
                        dst = fcl_g[64 * hl: 64 * hl + 64, :,
                                    12 * r + 6 * bb: 12 * r + 6 * bb + 6]
                        src = rt_v[64 * bb: 64 * bb + 64, bb, :, hl, :]
                        nc.vector.tensor_copy(out=dst, in_=src)

        PV_SCHED = {1: (2, 3), 2: (4, 5), 3: (6, 7)}
        for g in range(NQUAD):
            if g in PV_SCHED:  # issue next loads (px[g] already queued g-1)
                a, b = PV_SCHED[g]
                load_pv(a)
                load_pv(b)
                if g + 1 < NQUAD:
                    load_px(g + 1)

            ats = do_qk(g)

            if pend:
                finish_quad(pend)

            ax = ax_bufs[g % 2]
            axf = ax  # flat [128, 4992]
            axv = ax.rearrange("p (d i c) -> p d i c", d=NDC, i=2)
            do_exp(g, ats, axf)

            rsb = rs_ps.tile([96, 2 * PVW], F32, tag="rs", name=f"rsum{g}")
            rsum = [rsb[:, 0:PVW], rsb[:, PVW: 2 * PVW]]
            do_av(g, axv, rsum, 0, 10)
            pend = {"g": g, "axv": axv, "rsum": rsum}

        finish_quad(pend)

        # ---------------- tail: batched Wo projection ----------------
        o2 = rs_ps.tile([96, D], F32, tag="rs", name="o2")
        for kk in range(4):
            nc.tensor.matmul(
                out=o2, lhsT=fcl_g[:, kk, :],
                rhs=wo_sb[:, 192 * kk: 192 * kk + 192],
                start=(kk == 0), stop=(kk == 3),
            )
        out_sb = const.tile([96, D], F32)
        nc.vector.tensor_add(out=out_sb, in0=o2, in1=zz_sb)
        nc.sync.dma_start(out=out_h.ap(), in_=out_sb)

    return nc


def get_nc() -> bass.Bass:
    if "nc" not in _CACHE:
        nc = _build_nc()
        # The PJRT exec path serializes nc.m as-is; run Bacc's legalization
        # (wait splitting, register allocation, ...) explicitly.
        nc.finalize()
        _CACHE["nc"] = nc
    return _CACHE["nc"]


def make_in_maps(x, z, Wq, bq, Wo, bo):
    """Host-side prep + sharding into per-core input maps."""
    x = np.asarray(x, dtype=np.float32)
    z = np.asarray(z, dtype=np.float32)
    Wq = np.asarray(Wq, dtype=np.float32)
    bq = np.asarray(bq, dtype=np.float32)
    Wo = np.asarray(Wo, dtype=np.float32)
    bo = np.asarray(bo, dtype=np.float32)

    scale = np.float32(C ** -0.5)
    x_f8 = x.reshape(B, C, HW).astype(FP8)
    wq_s = (Wq * scale).astype(BF16)
    bq_s = (bq * scale).astype(BF16)
    wo_bf = Wo.astype(BF16)
    # pk2 = [ident 128 | wo 4*192] with wo[p, 192k+d] = Wo[128k+p, d]
    pk2 = np.zeros((128, 896), dtype=BF16)
    pk2[:, 0:128] = np.eye(128, dtype=BF16)
    pk2[:, 128:896] = np.ascontiguousarray(
        wo_bf.reshape(4, 128, D).transpose(1, 0, 2).reshape(128, 4 * D)
    )

    in_maps = []
    for ci in range(N_CORES):
        s = slice(ci * BPC, (ci + 1) * BPC)
        xc = x_f8[s]  # [16, 64, 3136]

        # px: QK stationary. px[g, 64bb+c, j, i, t] = x[4g+2i+bb, c, 128j+t]
        xp = np.zeros((BPC, C, NCHUNK, 128), dtype=FP8)
        xp[:, :, :24, :] = xc[:, :, : 24 * 128].reshape(BPC, C, 24, 128)
        xp[:, :, 24, :64] = xc[:, :, 24 * 128:]
        xq = xp.reshape(NQUAD, 2, 2, C, NCHUNK, 128)  # [g, i, bb, c, j, t]
        px = np.ascontiguousarray(xq.transpose(0, 2, 3, 4, 1, 5)).reshape(
            NQUAD * 128, NCHUNK * 2 * 128
        )

        # pv: AV moving (x^T with ones col).
        # pv[r, t, d, i, cc] = x[2r + cc//64, cc%64, 256d + 128i + t]
        xt_pad = np.zeros((NPAIR, NDC * 256, PVW), dtype=FP8)
        xt_pad[:, :HW, :128] = (
            xc.reshape(NPAIR, 2, C, HW).transpose(0, 3, 1, 2).reshape(NPAIR, HW, 128)
        )
        xt_pad[:, :HW, 128] = np.float32(1.0)
        pv = np.ascontiguousarray(
            xt_pad.reshape(NPAIR, NDC, 2, 128, PVW).transpose(0, 3, 1, 2, 4)
        ).reshape(NPAIR * 128, NDC * 2 * PVW)

        # zt[d, 6*b_local + m] = z[core_base + b_local, m, d]; bias folded via
        # ones row (zt1 row 64 = 1, wq1 row 64 = bq*scale)
        zt = z[s].reshape(BPC * M, D).T.astype(BF16)
        pk1 = np.zeros((128, 1216), dtype=BF16)
        pk1[:, 0:96] = zt[0:128]
        pk1[0:64, 96:192] = zt[128:192]
        pk1[64, 96:192] = np.float32(1.0)
        pk1[:, 192:704] = wq_s[0:128]
        pk1[0:64, 704:1216] = wq_s[128:192]
        pk1[64, 704:1216] = bq_s

        # zz[12r + 6bb + m] = z[2r + bb, m] + bo
        zz = (z[s] + bo[None, None, :]).reshape(96, D).astype(np.float32)

        in_maps.append({"px": px, "pv": pv, "pk1": pk1, "pk2": pk2, "zz": zz})
    return in_maps


def kernel(**inputs) -> np.ndarray:
    nc = get_nc()
    in_maps = make_in_maps(
        inputs["x"], inputs["z"], inputs["Wq"], inputs["bq"],
        inputs["Wo"], inputs["bo"],
    )
    res = run_bass_kernel_spmd(nc, in_maps, list(range(N_CORES)))
    out = np.concatenate(
        [
            np.asarray(res.results[i]["out"]).reshape(BPC, M, D)
            for i in range(N_CORES)
        ],
        axis=0,
    )
    return out.astype(np.float32)


# revision 16
# speedup vs baseline: 1.4066x; 1.1179x over previous
"""Trainium2 Bass kernel for Mobile2Former cross-attention block.

Computation (per batch b):
    xf   = x[b].reshape(C, H*W)                      # [64, 3136] keys=values
    q    = (z[b] @ Wq + bq).reshape(heads, M, C)     # [8, 6, 64]
    attn = softmax(q @ xf * C**-0.5, axis=-1)        # [8, 6, 3136]
    res  = attn @ xf.T                               # [8, 6, 64]
    out  = res.transpose(1,0,2).reshape(M, -1) @ Wo + bo + z[b]

Strategy: data-parallel over B across 8 cores (16 batches/core).  Batches are
processed in QUADS (4 batches) using fp8e4 DoubleRow matmuls: the contraction
dim packs two 128-row slabs (two batch-PAIRS block-diagonal for QK; two
consecutive 128-key chunks for AV), giving 2x tensor-engine throughput.  The
AV operand x^T arrives pre-transposed from the host (no on-chip transposes),
with a ones-column per slab yielding the softmax denominator for free.
Softmax runs without max subtraction (logits are O(1)).

The Scalar engine runs ONLY the exp waves (its ~20us of exp work is the
critical resource): per quad, 5 waves of 5 key-chunks land contiguously in
PSUM ([128, 960] f32; the one bank-crossing chunk is split into two matmuls)
so each exp is a single flat read.  The per-quad PE program runs QK waves
0-2, then the whole deferred AV/normalize of the previous quad, then waves
3-4, so the PE always has runnable work ahead of each exp-paced wait and exp
never stalls.  The q projection folds bias via a ones-row in the stationary;
one bulk DVE f32->fp8 conversion plus 8 small SBUF copies (vector+gpsimd)
build the block-diagonal moving operand.  All DMA issue is on sync; the Wo
projection runs 3/4 early, with only pairs 6-7 on the critical tail.
"""

import sys
from contextlib import ExitStack

import numpy as np

sys.path.insert(0, "/opt/trn_rl_repo")

import concourse.bass as bass
import concourse.tile as tile
from concourse import bacc as bacc_mod
from concourse import mybir
from concourse.bass_utils import run_bass_kernel_spmd

import ml_dtypes

BF16 = ml_dtypes.bfloat16
FP8 = ml_dtypes.float8_e4m3

N_CORES = 8
B, C, H, W = 128, 64, 56, 56
HW = H * W  # 3136
M, D = 6, 192
NH = 8
INNER = NH * C  # 512
BPC = B // N_CORES  # 16 batches per core
NQUAD = BPC // 4  # 4
NPAIR = BPC // 2  # 8
NCHUNK = (HW + 127) // 128  # 25 (24 full + one 64-wide)
NDC = (NCHUNK + 1) // 2  # 13 double-chunks (last has a dead slab)
PVW = 132  # AV moving cols per slab: 128 c + 1 ones + 3 pad

F32 = mybir.dt.float32
BF = mybir.dt.bfloat16
F8 = mybir.dt.float8e4
DR = mybir.MatmulPerfMode.DoubleRow
EXP = mybir.ActivationFunctionType.Exp

_CACHE = {}


def _build_nc() -> bass.Bass:
    nc = bacc_mod.Bacc()

    # QK stationary: [quad*128 part (2b x 64c), 25 j, 2 slab(pair), 128 n] fp8
    px_h = nc.declare_dram_parameter("px", [NQUAD * 128, NCHUNK * 2 * 128], F8,
                                     isOutput=False)
    # AV moving: [pair*128 part (n%128), 13 dc, 2 slab(n chunk), 132] fp8
    pv_h = nc.declare_dram_parameter("pv", [NPAIR * 128, NDC * 2 * PVW], F8,
                                     isOutput=False)
    # pk1 cols: [zt0 96][zt1+ones 96][wq0 512][wq1+bq 512] (rows 0:65 for 1-blks)
    pk1_h = nc.declare_dram_parameter("pk1", [128, 1216], BF, isOutput=False)
    # pk2 cols: [ident_bf 128][wo 4*192]
    pk2_h = nc.declare_dram_parameter("pk2", [128, 896], BF, isOutput=False)
    # z + bo in output layout: row 12r + 6bb + m
    zz_h = nc.declare_dram_parameter("zz", [96, D], F32, isOutput=False)
    out_h = nc.declare_dram_parameter("out", [96, D], F32, isOutput=True)

    with tile.TileContext(nc) as tc, ExitStack() as ctx:
        const = ctx.enter_context(tc.tile_pool(name="const", bufs=1))
        px_pool = ctx.enter_context(tc.tile_pool(name="px", bufs=3))
        pv_pool = ctx.enter_context(tc.tile_pool(name="pv", bufs=4))
        small = ctx.enter_context(tc.tile_pool(name="small", bufs=3))
        at_ps = ctx.enter_context(tc.tile_pool(name="at_ps", bufs=3, space="PSUM"))
        rs_ps = ctx.enter_context(tc.tile_pool(name="rs_ps", bufs=2, space="PSUM"))

        # ---------------- phase 0: warmup + constants ----------------
        # Pre-warm the Exp table; the two big probes also A/B fp8-vs-bf16
        # activation output rate in the trace (scalar is idle this early).
        warm = const.tile([128, 960], F32)
        nc.vector.memset(warm, 0.0)
        warm_f8 = const.tile([128, 960], F8)
        nc.scalar.activation(out=warm_f8, in_=warm, func=EXP)
        warm_bf = const.tile([128, 960], BF)
        nc.scalar.activation(out=warm_bf, in_=warm, func=EXP)

        pk1 = const.tile([128, 1216], BF)
        nc.sync.dma_start(out=pk1, in_=pk1_h.ap())
        zt0 = pk1[:, 0:96]
        zt1 = pk1[0:65, 96:192]
        wq0 = pk1[:, 192:704]
        wq1 = pk1[0:65, 704:1216]

        px_t = []
        for g in range(NQUAD):
            px_t.append(px_pool.tile([128, NCHUNK * 2 * 128], F8, tag="px",
                                     name=f"px{g}"))
        pv_t = []
        for r in range(NPAIR):
            pv_t.append(pv_pool.tile([128, NDC * 2 * PVW], F8, tag="pv",
                                     name=f"pv{r}"))

        def load_px(g):
            nc.sync.dma_start(out=px_t[g], in_=px_h.ap()[128 * g: 128 * (g + 1), :])

        def load_pv(r):
            nc.sync.dma_start(out=pv_t[r], in_=pv_h.ap()[128 * r: 128 * (r + 1), :])

        load_px(0)
        pk2 = const.tile([128, 896], BF)
        nc.sync.dma_start(out=pk2, in_=pk2_h.ap())
        ident_bf = pk2[:, 0:128]
        wo_sb = pk2[:, 128:896]
        zz_sb = const.tile([96, D], F32)
        nc.sync.dma_start(out=zz_sb, in_=zz_h.ap())
        load_pv(0)
        load_pv(1)
        load_px(1)

        # ---------------- qproj ----------------
        # qp[64gh+c, 96ii + 6bl + t] = q^T[128ii + 64gh + c, batch bl, row t]
        # (bias folded: zt1 row 64 = ones, wq1 row 64 = bq*scale)
        qp = at_ps.tile([128, 4 * 96], F32, tag="at", name="qp")
        qp_g = qp.rearrange("p (ii x) -> p ii x", ii=4)
        for ii in range(4):
            nc.tensor.matmul(
                qp_g[:, ii, :], lhsT=wq0[:, 128 * ii: 128 * ii + 128], rhs=zt0,
                start=True, stop=False,
            )
            nc.tensor.matmul(
                qp_g[:, ii, :], lhsT=wq1[:, 128 * ii: 128 * ii + 128], rhs=zt1,
                start=False, stop=True,
            )
        # bulk f32 -> fp8 conversion, then 8 small SBUF shuffles
        qa = const.tile([128, 384], F8)
        nc.vector.tensor_copy(out=qa, in_=qp)
        qa_r = qa.rearrange("p (ii g i b2 t) -> p g t ii i b2",
                            ii=4, g=NQUAD, i=2, b2=2)

        # qT4big: QK moving operand, [128 (c2), 4 g, 2 slab(pair), 192] fp8;
        # valid block of quad g, slab i: cols 96i:96i+96 (block-diagonal).
        # In-block col = 48bb + 8t + 2ii + gh; flat offset within a g-block is
        # 288i + 48bb + gh + 8t + 2ii.
        qT4big = const.tile([128, NQUAD * 2 * 192], F8)
        nc.gpsimd.memset(qT4big, 0.0)
        qT4_gv = qT4big.rearrange("p (g y) -> p g y", g=NQUAD)
        for i in range(2):
            for bb in range(2):
                for gh in range(2):
                    base = 288 * i + 48 * bb
                    dst = qT4_gv[64 * bb: 64 * bb + 64, :, base: base + 48]
                    dst = dst.rearrange("p g (t ii w) -> p g t ii w", t=6, ii=4)
                    dst = dst[:, :, :, :, gh]
                    src = qa_r[64 * gh: 64 * gh + 64, :, :, :, i, bb]
                    eng = nc.vector if i == 0 else nc.gpsimd
                    eng.tensor_copy(out=dst, in_=src)
        qT4v_all = qT4big.rearrange("p (g i c) -> p g i c", g=NQUAD, i=2)

        # ax buffers: exp output / AV stationary, [128, 13 dc, 2 slab, 192] fp8.
        # Dead tail region (dc12 slab1) pre-zeroed once; exp never writes it.
        ax_bufs = []
        for i in range(2):
            t = const.tile([128, NDC * 2 * 192], F8, name=f"ax_buf{i}")
            tv = t.rearrange("p (d i c) -> p d i c", d=NDC, i=2)
            nc.gpsimd.memset(tv[64:128, NDC - 1, 0, :], 0.0)
            nc.gpsimd.memset(tv[:, NDC - 1, 1, :], 0.0)
            ax_bufs.append(t)

        # fcl_all: Wo-projection stationary for all 8 pairs,
        # fcl_all[64*hl + c, kk, 12*r + 6*bb + m] bf16
        fcl_all = const.tile([128, 4 * 96], BF)
        fcl_g = fcl_all.rearrange("q (kk x) -> q kk x", kk=4)
        out_sb = const.tile([96, D], F32)

        # ---------------- per-quad pieces ----------------
        def do_qk_waves(g, ats, waves):
            pxv = px_t[g].rearrange("p (j i t) -> p j i t", j=NCHUNK, i=2)
            qT4v = qT4v_all[:, g, :, :]
            for w in waves:
                at = at_ps.tile([128, 960], F32, tag="at", name=f"at{g}_{w}")
                ats[w] = at
                for jj in range(5):
                    j = 5 * w + jj
                    cw = 64 if j == NCHUNK - 1 else 128
                    if jj == 2:  # split at the PSUM bank boundary (el 512)
                        nc.tensor.matmul(
                            at[0:cw, 384:512], lhsT=pxv[:, j, :, 0:cw],
                            rhs=qT4v[:, :, 0:128], perf_mode=DR,
                            start=True, stop=True,
                        )
                        nc.tensor.matmul(
                            at[0:cw, 512:576], lhsT=pxv[:, j, :, 0:cw],
                            rhs=qT4v[:, :, 128:192], perf_mode=DR,
                            start=True, stop=True,
                        )
                    else:
                        o = 192 * jj
                        nc.tensor.matmul(
                            at[0:cw, o: o + 192], lhsT=pxv[:, j, :, 0:cw],
                            rhs=qT4v, perf_mode=DR, start=True, stop=True,
                        )

        def do_exp(g, ats, axf):
            for w in range(5):
                nc.scalar.activation(
                    out=axf[:, 960 * w: 960 * (w + 1)], in_=ats[w], func=EXP,
                )

        def do_av_all(p):
            g, axv, rsum = p["g"], p["axv"], p["rsum"]
            for d in range(NDC):
                for i in range(2):
                    nc.tensor.matmul(
                        rsum[i], lhsT=axv[:, d, :, 96 * i: 96 * i + 96],
                        rhs=pv_t[2 * g + i].rearrange(
                            "p (d i c) -> p d i c", d=NDC, i=2)[:, d, :, :],
                        perf_mode=DR, start=(d == 0), stop=(d == NDC - 1),
                    )

        def do_norm(p):
            # normalize both pairs, transpose into one tile, 4 merged fcl
            # copies: fcl[64hl+c, kk, 12r+6bb+m] = rtb[64bb+c, 96i+48bb+12kk+6hl+m]
            g, rsum = p["g"], p["rsum"]
            rtb = rs_ps.tile([128, 2 * 96], BF, tag="rs", name=f"rtb{g}")
            for i in range(2):
                r = 2 * g + i
                inv = small.tile([96, 1], F32, tag="inv", name=f"inv{r}")
                nc.vector.reciprocal(out=inv, in_=rsum[i][:, 128:129])
                r2n = small.tile([96, 128], BF, tag="r2n", name=f"r2n{r}")
                nc.vector.tensor_scalar_mul(out=r2n, in0=rsum[i][:, 0:128],
                                            scalar1=inv)
                nc.tensor.transpose(rtb[:, 96 * i: 96 * i + 96], r2n,
                                    ident_bf[0:96, 0:96])
            rt_v = rtb.rearrange("q (i b2 kk h2 m) -> q kk i b2 h2 m",
                                 i=2, b2=2, kk=4, h2=2)
            for hl in range(2):
                for bb in range(2):
                    dst = fcl_g[64 * hl: 64 * hl + 64, :, 24 * g: 24 * g + 24]
                    dst = dst.rearrange("p kk (i b2 m) -> p kk i b2 m",
                                        i=2, b2=2)[:, :, :, bb, :]
                    src = rt_v[64 * bb: 64 * bb + 64, :, :, bb, hl, :]
                    nc.vector.tensor_copy(out=dst, in_=src)

        def do_wo(r0, nr, o2):
            # out rows r0 .. r0+nr of the Wo projection + residual
            sl = slice(r0, r0 + nr)
            for kk in range(4):
                nc.tensor.matmul(
                    out=o2, lhsT=fcl_g[:, kk, sl],
                    rhs=wo_sb[:, 192 * kk: 192 * kk + 192],
                    start=(kk == 0), stop=(kk == 3),
                )
            nc.vector.tensor_add(out=out_sb[sl, :], in0=o2, in1=zz_sb[sl, :])
            nc.sync.dma_start(out=out_h.ap()[sl, :], in_=out_sb[sl, :])

        # ---------------- main loop ----------------
        PV_SCHED = {1: (2, 3), 2: (4, 5), 3: (6, 7)}
        pend = {}
        for g in range(NQUAD):
            if g in PV_SCHED:
                a, b = PV_SCHED[g]
                load_pv(a)
                load_pv(b)
                if g + 1 < NQUAD:
                    load_px(g + 1)

            ats = {}
            do_qk_waves(g, ats, [0, 1])
            if pend:
                do_av_all(pend)
            do_qk_waves(g, ats, [2, 3, 4])
            if pend:
                do_norm(pend)
                if pend["g"] == 2:
                    o2a = rs_ps.tile([64, D], F32, tag="rs", name="o2a")
                    do_wo(0, 64, o2a)

            ax = ax_bufs[g % 2]
            axv = ax.rearrange("p (d i c) -> p d i c", d=NDC, i=2)
            do_exp(g, ats, ax)

            rsb = rs_ps.tile([96, 2 * PVW], F32, tag="rs", name=f"rsum{g}")
            pend = {"g": g, "axv": axv,
                    "rsum": [rsb[:, 0:PVW], rsb[:, PVW: 2 * PVW]]}

        do_av_all(pend)
        do_norm(pend)
        o2b = rs_ps.tile([32, D], F32, tag="rs", name="o2b")
        do_wo(64, 32, o2b)

    return nc


def get_nc() -> bass.Bass:
    if "nc" not in _CACHE:
        nc = _build_nc()
        # The PJRT exec path serializes nc.m as-is; run Bacc's legalization
        # (wait splitting, register allocation, ...) explicitly.
        nc.finalize()
        _CACHE["nc"] = nc
    return _CACHE["nc"]


def make_in_maps(x, z, Wq, bq, Wo, bo):
    """Host-side prep + sharding into per-core input maps."""
    x = np.asarray(x, dtype=np.float32)
    z = np.asarray(z, dtype=np.float32)
    Wq = np.asarray(Wq, dtype=np.float32)
    bq = np.asarray(bq, dtype=np.float32)
    Wo = np.asarray(Wo, dtype=np.float32)
    bo = np.asarray(bo, dtype=np.float32)

    scale = np.float32(C ** -0.5)
    x_f8 = x.reshape(B, C, HW).astype(FP8)
    wq_s = (Wq * scale).astype(BF16)
    bq_s = (bq * scale).astype(BF16)
    wo_bf = Wo.astype(BF16)
    # pk2 = [ident 128 | wo 4*192] with wo[p, 192k+d] = Wo[128k+p, d]
    pk2 = np.zeros((128, 896), dtype=BF16)
    pk2[:, 0:128] = np.eye(128, dtype=BF16)
    pk2[:, 128:896] = np.ascontiguousarray(
        wo_bf.reshape(4, 128, D).transpose(1, 0, 2).reshape(128, 4 * D)
    )

    in_maps = []
    for ci in range(N_CORES):
        s = slice(ci * BPC, (ci + 1) * BPC)
        xc = x_f8[s]  # [16, 64, 3136]

        # px: QK stationary. px[g, 64bb+c, j, i, t] = x[4g+2i+bb, c, 128j+t]
        xp = np.zeros((BPC, C, NCHUNK, 128), dtype=FP8)
        xp[:, :, :24, :] = xc[:, :, : 24 * 128].reshape(BPC, C, 24, 128)
        xp[:, :, 24, :64] = xc[:, :, 24 * 128:]
        xq = xp.reshape(NQUAD, 2, 2, C, NCHUNK, 128)  # [g, i, bb, c, j, t]
        px = np.ascontiguousarray(xq.transpose(0, 2, 3, 4, 1, 5)).reshape(
            NQUAD * 128, NCHUNK * 2 * 128
        )

        # pv: AV moving (x^T with ones col).
        # pv[r, t, d, i, cc] = x[2r + cc//64, cc%64, 256d + 128i + t]
        xt_pad = np.zeros((NPAIR, NDC * 256, PVW), dtype=FP8)
        xt_pad[:, :HW, :128] = (
            xc.reshape(NPAIR, 2, C, HW).transpose(0, 3, 1, 2).reshape(NPAIR, HW, 128)
        )
        xt_pad[:, :HW, 128] = np.float32(1.0)
        pv = np.ascontiguousarray(
            xt_pad.reshape(NPAIR, NDC, 2, 128, PVW).transpose(0, 3, 1, 2, 4)
        ).reshape(NPAIR * 128, NDC * 2 * PVW)

        # zt[d, 6*b_local + m] = z[core_base + b_local, m, d]; bias folded via
        # ones row (zt1 row 64 = 1, wq1 row 64 = bq*scale)
        zt = z[s].reshape(BPC * M, D).T.astype(BF16)
        pk1 = np.zeros((128, 1216), dtype=BF16)
        pk1[:, 0:96] = zt[0:128]
        pk1[0:64, 96:192] = zt[128:192]
        pk1[64, 96:192] = np.float32(1.0)
        pk1[:, 192:704] = wq_s[0:128]
        pk1[0:64, 704:1216] = wq_s[128:192]
        pk1[64, 704:1216] = bq_s

        # zz[12r + 6bb + m] = z[2r + bb, m] + bo
        zz = (z[s] + bo[None, None, :]).reshape(96, D).astype(np.float32)

        in_maps.append({"px": px, "pv": pv, "pk1": pk1, "pk2": pk2, "zz": zz})
    return in_maps


def kernel(**inputs) -> np.ndarray:
    nc = get_nc()
    in_maps = make_in_maps(
        inputs["x"], inputs["z"], inputs["Wq"], inputs["bq"],
        inputs["Wo"], inputs["bo"],
    )
    res = run_bass_kernel_spmd(nc, in_maps, list(range(N_CORES)))
    out = np.concatenate(
        [
            np.asarray(res.results[i]["out"]).reshape(BPC, M, D)
            for i in range(N_CORES)
        ],
        axis=0,
    )
    return out.astype(np.float32)


# revision 19
# speedup vs baseline: 1.5243x; 1.0836x over previous
"""Trainium2 Bass kernel for Mobile2Former cross-attention block.

Computation (per batch b):
    xf   = x[b].reshape(C, H*W)                      # [64, 3136] keys=values
    q    = (z[b] @ Wq + bq).reshape(heads, M, C)     # [8, 6, 64]
    attn = softmax(q @ xf * C**-0.5, axis=-1)        # [8, 6, 3136]
    res  = attn @ xf.T                               # [8, 6, 64]
    out  = res.transpose(1,0,2).reshape(M, -1) @ Wo + bo + z[b]

Strategy: data-parallel over B across 8 cores (16 batches/core).  Batches are
processed in QUADS (4 batches) using fp8e4 DoubleRow matmuls: the contraction
dim packs two 128-row slabs (two batch-PAIRS block-diagonal for QK; two
consecutive 128-key chunks for AV), giving 2x tensor-engine throughput.  The
AV operand x^T arrives pre-transposed from the host (no on-chip transposes),
with a ones-column per slab yielding the softmax denominator for free.
Softmax runs without max subtraction (logits are O(1)).

The Scalar engine runs ONLY the exp waves (its ~20us of exp work is the
critical resource): per quad, 5 waves of 5 key-chunks land contiguously in
PSUM ([128, 960] f32; the one bank-crossing chunk is split into two matmuls)
so each exp is a single flat read.  The per-quad PE program runs QK waves
0-2, then the whole deferred AV/normalize of the previous quad, then waves
3-4, so the PE always has runnable work ahead of each exp-paced wait and exp
never stalls.  The q projection folds bias via a ones-row in the stationary;
one bulk DVE f32->fp8 conversion plus 8 small SBUF copies (vector+gpsimd)
build the block-diagonal moving operand.  All DMA issue is on sync; the Wo
projection runs 3/4 early, with only pairs 6-7 on the critical tail.
"""

import sys
from contextlib import ExitStack

import numpy as np

sys.path.insert(0, "/opt/trn_rl_repo")

import concourse.bass as bass
import concourse.tile as tile
from concourse import bacc as bacc_mod
from concourse import mybir
from concourse.bass_utils import run_bass_kernel_spmd

import ml_dtypes

BF16 = ml_dtypes.bfloat16
FP8 = ml_dtypes.float8_e4m3

N_CORES = 8
B, C, H, W = 128, 64, 56, 56
HW = H * W  # 3136
M, D = 6, 192
NH = 8
INNER = NH * C  # 512
BPC = B // N_CORES  # 16 batches per core
NQUAD = BPC // 4  # 4
NPAIR = BPC // 2  # 8
NCHUNK = (HW + 127) // 128  # 25 (24 full + one 64-wide)
NDC = (NCHUNK + 1) // 2  # 13 double-chunks (last has a dead slab)
PVW = 132  # AV moving cols per slab: 128 c + 1 ones + 3 pad

F32 = mybir.dt.float32
BF = mybir.dt.bfloat16
F8 = mybir.dt.float8e4
DR = mybir.MatmulPerfMode.DoubleRow
EXP = mybir.ActivationFunctionType.Exp

_CACHE = {}


def _build_nc() -> bass.Bass:
    nc = bacc_mod.Bacc()

    # QK stationary: [quad*128 part (2b x 64c), 25 j, 2 slab(pair), 128 n] fp8
    px_h = nc.declare_dram_parameter("px", [NQUAD * 128, NCHUNK * 2 * 128], F8,
                                     isOutput=False)
    # AV moving: [pair*128 part (n%128), 13 dc, 2 slab(n chunk), 132] fp8
    pv_h = nc.declare_dram_parameter("pv", [NPAIR * 128, NDC * 2 * PVW], F8,
                                     isOutput=False)
    # pk1 cols: [zt0 96][zt1+ones 96][wq0 512][wq1+bq 512] (rows 0:65 for 1-blks)
    pk1_h = nc.declare_dram_parameter("pk1", [128, 1216], BF, isOutput=False)
    # pk2 cols: [ident_bf 128][wo 4*192]
    pk2_h = nc.declare_dram_parameter("pk2", [128, 896], BF, isOutput=False)
    # z + bo in output layout: row 12r + 6bb + m
    zz_h = nc.declare_dram_parameter("zz", [96, D], F32, isOutput=False)
    out_h = nc.declare_dram_parameter("out", [96, D], F32, isOutput=True)

    with tile.TileContext(nc) as tc, ExitStack() as ctx:
        const = ctx.enter_context(tc.tile_pool(name="const", bufs=1))
        px_pool = ctx.enter_context(tc.tile_pool(name="px", bufs=3))
        pv_pool = ctx.enter_context(tc.tile_pool(name="pv", bufs=4))
        small = ctx.enter_context(tc.tile_pool(name="small", bufs=3))
        at_ps = ctx.enter_context(tc.tile_pool(name="at_ps", bufs=3, space="PSUM"))
        rs_ps = ctx.enter_context(tc.tile_pool(name="rs_ps", bufs=2, space="PSUM"))

        # ---------------- phase 0: warmup + constants ----------------
        # Tiny exp to trigger the ACT table load early (scalar idle til then).
        warm = const.tile([128, 8], F32)
        nc.vector.memset(warm, 0.0)
        warm2 = const.tile([128, 8], F32)
        nc.scalar.activation(out=warm2, in_=warm, func=EXP)

        # pk1 rides alone on the sync queue so qproj's inputs land first; the
        # quad-0 x loads use the (otherwise idle pre-exp) scalar HWDGE queue.
        pk1 = const.tile([128, 1216], BF)
        nc.sync.dma_start(out=pk1, in_=pk1_h.ap())
        zt0 = pk1[:, 0:96]
        zt1 = pk1[0:65, 96:192]
        wq0 = pk1[:, 192:704]
        wq1 = pk1[0:65, 704:1216]

        px_t = []
        for g in range(NQUAD):
            px_t.append(px_pool.tile([128, NCHUNK * 2 * 128], F8, tag="px",
                                     name=f"px{g}"))
        pv_t = []
        for r in range(NPAIR):
            pv_t.append(pv_pool.tile([128, NDC * 2 * PVW], F8, tag="pv",
                                     name=f"pv{r}"))

        def load_px(g, eng=None):
            (eng or nc.sync).dma_start(
                out=px_t[g], in_=px_h.ap()[128 * g: 128 * (g + 1), :])

        def load_pv(r, eng=None):
            (eng or nc.sync).dma_start(
                out=pv_t[r], in_=pv_h.ap()[128 * r: 128 * (r + 1), :])

        load_px(0, nc.scalar)
        load_pv(0, nc.scalar)
        load_pv(1, nc.scalar)
        pk2 = const.tile([128, 896], BF)
        nc.sync.dma_start(out=pk2, in_=pk2_h.ap())
        ident_bf = pk2[:, 0:128]
        wo_sb = pk2[:, 128:896]
        zz_sb = const.tile([96, D], F32)
        nc.sync.dma_start(out=zz_sb, in_=zz_h.ap())
        load_px(1)

        # ---------------- qproj ----------------
        # qp[64gh+c, 96ii + 6bl + t] = q^T[128ii + 64gh + c, batch bl, row t]
        # (bias folded: zt1 row 64 = ones, wq1 row 64 = bq*scale)
        qp = rs_ps.tile([128, 4 * 96], F32, tag="rs", name="qp")
        qp_g = qp.rearrange("p (ii x) -> p ii x", ii=4)
        for ii in range(4):
            nc.tensor.matmul(
                qp_g[:, ii, :], lhsT=wq0[:, 128 * ii: 128 * ii + 128], rhs=zt0,
                start=True, stop=False,
            )
            nc.tensor.matmul(
                qp_g[:, ii, :], lhsT=wq1[:, 128 * ii: 128 * ii + 128], rhs=zt1,
                start=False, stop=True,
            )
        # bulk f32 -> fp8 conversion (split by partition half so the gh=0
        # shuffles can start early), then small SBUF shuffles
        qa = const.tile([128, 384], F8)
        nc.vector.tensor_copy(out=qa[0:64, :], in_=qp[0:64, :])
        nc.vector.tensor_copy(out=qa[64:128, :], in_=qp[64:128, :])
        qa_r = qa.rearrange("p (ii g i b2 t) -> p g t ii i b2",
                            ii=4, g=NQUAD, i=2, b2=2)

        # qT4big: QK moving operand, [128 (c2), 4 g, 2 slab(pair), 192] fp8;
        # valid block of quad g, slab i: cols 96i:96i+96 (block-diagonal).
        # In-block col = 48bb + 8t + 2ii + gh; flat offset within a g-block is
        # 288i + 48bb + gh + 8t + 2ii.
        qT4big = const.tile([128, NQUAD * 2 * 192], F8)
        nc.gpsimd.memset(qT4big, 0.0)
        qT4_gv = qT4big.rearrange("p (g y) -> p g y", g=NQUAD)
        for gs in range(2):  # quads 0-1 first so QK can start early
            for i in range(2):
                for bb in range(2):
                    for gh in range(2):
                        base = 288 * i + 48 * bb
                        dst = qT4_gv[64 * bb: 64 * bb + 64, 2 * gs: 2 * gs + 2,
                                     base: base + 48]
                        dst = dst.rearrange("p g (t ii w) -> p g t ii w",
                                            t=6, ii=4)
                        dst = dst[:, :, :, :, gh]
                        src = qa_r[64 * gh: 64 * gh + 64, 2 * gs: 2 * gs + 2,
                                   :, :, i, bb]
                        eng = nc.gpsimd if gh == 0 else nc.vector
                        eng.tensor_copy(out=dst, in_=src)
        qT4v_all = qT4big.rearrange("p (g i c) -> p g i c", g=NQUAD, i=2)

        # ax buffers: exp output / AV stationary, [128, 13 dc, 2 slab, 192] fp8.
        # Dead tail region (dc12 slab1) pre-zeroed once; exp never writes it.
        ax_bufs = []
        for i in range(2):
            t = const.tile([128, NDC * 2 * 192], F8, name=f"ax_buf{i}")
            tv = t.rearrange("p (d i c) -> p d i c", d=NDC, i=2)
            nc.gpsimd.memset(tv[64:128, NDC - 1, 0, :], 0.0)
            nc.gpsimd.memset(tv[:, NDC - 1, 1, :], 0.0)
            ax_bufs.append(t)

        # fcl_all: Wo-projection stationary for all 8 pairs,
        # fcl_all[64*hl + c, kk, 12*r + 6*bb + m] bf16
        fcl_all = const.tile([128, 4 * 96], BF)
        fcl_g = fcl_all.rearrange("q (kk x) -> q kk x", kk=4)
        out_sb = const.tile([96, D], F32)

        # ---------------- per-quad pieces ----------------
        def do_qk_waves(g, ats, waves):
            pxv = px_t[g].rearrange("p (j i t) -> p j i t", j=NCHUNK, i=2)
            qT4v = qT4v_all[:, g, :, :]
            for w in waves:
                at = at_ps.tile([128, 960], F32, tag="at", name=f"at{g}_{w}")
                ats[w] = at
                for jj in range(5):
                    j = 5 * w + jj
                    cw = 64 if j == NCHUNK - 1 else 128
                    if jj == 2:  # split at the PSUM bank boundary (el 512)
                        nc.tensor.matmul(
                            at[0:cw, 384:512], lhsT=pxv[:, j, :, 0:cw],
                            rhs=qT4v[:, :, 0:128], perf_mode=DR,
                            start=True, stop=True,
                        )
                        nc.tensor.matmul(
                            at[0:cw, 512:576], lhsT=pxv[:, j, :, 0:cw],
                            rhs=qT4v[:, :, 128:192], perf_mode=DR,
                            start=True, stop=True,
                        )
                    else:
                        o = 192 * jj
                        nc.tensor.matmul(
                            at[0:cw, o: o + 192], lhsT=pxv[:, j, :, 0:cw],
                            rhs=qT4v, perf_mode=DR, start=True, stop=True,
                        )

        def do_exp(g, ats, axf):
            for w in range(5):
                nc.scalar.activation(
                    out=axf[:, 960 * w: 960 * (w + 1)], in_=ats[w], func=EXP,
                )

        def do_av_all(p):
            g, axv, rsum = p["g"], p["axv"], p["rsum"]
            for d in range(NDC):
                for i in range(2):
                    nc.tensor.matmul(
                        rsum[i], lhsT=axv[:, d, :, 96 * i: 96 * i + 96],
                        rhs=pv_t[2 * g + i].rearrange(
                            "p (d i c) -> p d i c", d=NDC, i=2)[:, d, :, :],
                        perf_mode=DR, start=(d == 0), stop=(d == NDC - 1),
                    )

        def do_norm(p):
            # normalize both pairs, transpose into one tile, 4 merged fcl
            # copies: fcl[64hl+c, kk, 12r+6bb+m] = rtb[64bb+c, 96i+48bb+12kk+6hl+m]
            g, rsum = p["g"], p["rsum"]
            rtb = rs_ps.tile([128, 2 * 96], BF, tag="rs", name=f"rtb{g}")
            for i in range(2):
                r = 2 * g + i
                inv = small.tile([96, 1], F32, tag="inv", name=f"inv{r}")
                nc.vector.reciprocal(out=inv, in_=rsum[i][:, 128:129])
                r2n = small.tile([96, 128], BF, tag="r2n", name=f"r2n{r}")
                nc.vector.tensor_scalar_mul(out=r2n, in0=rsum[i][:, 0:128],
                                            scalar1=inv)
                nc.tensor.transpose(rtb[:, 96 * i: 96 * i + 96], r2n,
                                    ident_bf[0:96, 0:96])
            rt_v = rtb.rearrange("q (i b2 kk h2 m) -> q kk i b2 h2 m",
                                 i=2, b2=2, kk=4, h2=2)
            for hl in range(2):
                for bb in range(2):
                    dst = fcl_g[64 * hl: 64 * hl + 64, :, 24 * g: 24 * g + 24]
                    dst = dst.rearrange("p kk (i b2 m) -> p kk i b2 m",
                                        i=2, b2=2)[:, :, :, bb, :]
                    src = rt_v[64 * bb: 64 * bb + 64, :, :, bb, hl, :]
                    nc.vector.tensor_copy(out=dst, in_=src)

        def do_wo(r0, nr, o2):
            # out rows r0 .. r0+nr of the Wo projection + residual
            sl = slice(r0, r0 + nr)
            for kk in range(4):
                nc.tensor.matmul(
                    out=o2, lhsT=fcl_g[:, kk, sl],
                    rhs=wo_sb[:, 192 * kk: 192 * kk + 192],
                    start=(kk == 0), stop=(kk == 3),
                )
            nc.vector.tensor_add(out=out_sb[sl, :], in0=o2, in1=zz_sb[sl, :])
            nc.sync.dma_start(out=out_h.ap()[sl, :], in_=out_sb[sl, :])

        # ---------------- main loop ----------------
        PV_SCHED = {1: (2, 3), 2: (4, 5), 3: (6, 7)}
        pend = {}
        for g in range(NQUAD):
            if g in PV_SCHED:
                a, b = PV_SCHED[g]
                load_pv(a)
                load_pv(b)
                if g + 1 < NQUAD:
                    load_px(g + 1)

            ats = {}
            do_qk_waves(g, ats, [0, 1])
            if pend:
                do_av_all(pend)
            do_qk_waves(g, ats, [2, 3, 4])
            if pend:
                do_norm(pend)
                if pend["g"] == 2:
                    o2a = rs_ps.tile([64, D], F32, tag="rs", name="o2a")
                    do_wo(0, 64, o2a)

            ax = ax_bufs[g % 2]
            axv = ax.rearrange("p (d i c) -> p d i c", d=NDC, i=2)
            do_exp(g, ats, ax)

            rsb = rs_ps.tile([96, 2 * PVW], F32, tag="rs", name=f"rsum{g}")
            pend = {"g": g, "axv": axv,
                    "rsum": [rsb[:, 0:PVW], rsb[:, PVW: 2 * PVW]]}

        do_av_all(pend)
        do_norm(pend)
        o2b = rs_ps.tile([32, D], F32, tag="rs", name="o2b")
        do_wo(64, 32, o2b)

    return nc


def get_nc() -> bass.Bass:
    if "nc" not in _CACHE:
        nc = _build_nc()
        # The PJRT exec path serializes nc.m as-is; run Bacc's legalization
        # (wait splitting, register allocation, ...) explicitly.
        nc.finalize()
        _CACHE["nc"] = nc
    return _CACHE["nc"]


def make_in_maps(x, z, Wq, bq, Wo, bo):
    """Host-side prep + sharding into per-core input maps."""
    x = np.asarray(x, dtype=np.float32)
    z = np.asarray(z, dtype=np.float32)
    Wq = np.asarray(Wq, dtype=np.float32)
    bq = np.asarray(bq, dtype=np.float32)
    Wo = np.asarray(Wo, dtype=np.float32)
    bo = np.asarray(bo, dtype=np.float32)

    scale = np.float32(C ** -0.5)
    x_f8 = x.reshape(B, C, HW).astype(FP8)
    wq_s = (Wq * scale).astype(BF16)
    bq_s = (bq * scale).astype(BF16)
    wo_bf = Wo.astype(BF16)
    # pk2 = [ident 128 | wo 4*192] with wo[p, 192k+d] = Wo[128k+p, d]
    pk2 = np.zeros((128, 896), dtype=BF16)
    pk2[:, 0:128] = np.eye(128, dtype=BF16)
    pk2[:, 128:896] = np.ascontiguousarray(
        wo_bf.reshape(4, 128, D).transpose(1, 0, 2).reshape(128, 4 * D)
    )

    in_maps = []
    for ci in range(N_CORES):
        s = slice(ci * BPC, (ci + 1) * BPC)
        xc = x_f8[s]  # [16, 64, 3136]

        # px: QK stationary. px[g, 64bb+c, j, i, t] = x[4g+2i+bb, c, 128j+t]
        xp = np.zeros((BPC, C, NCHUNK, 128), dtype=FP8)
        xp[:, :, :24, :] = xc[:, :, : 24 * 128].reshape(BPC, C, 24, 128)
        xp[:, :, 24, :64] = xc[:, :, 24 * 128:]
        xq = xp.reshape(NQUAD, 2, 2, C, NCHUNK, 128)  # [g, i, bb, c, j, t]
        px = np.ascontiguousarray(xq.transpose(0, 2, 3, 4, 1, 5)).reshape(
            NQUAD * 128, NCHUNK * 2 * 128
        )

        # pv: AV moving (x^T with ones col).
        # pv[r, t, d, i, cc] = x[2r + cc//64, cc%64, 256d + 128i + t]
        xt_pad = np.zeros((NPAIR, NDC * 256, PVW), dtype=FP8)
        xt_pad[:, :HW, :128] = (
            xc.reshape(NPAIR, 2, C, HW).transpose(0, 3, 1, 2).reshape(NPAIR, HW, 128)
        )
        xt_pad[:, :HW, 128] = np.float32(1.0)
        pv = np.ascontiguousarray(
            xt_pad.reshape(NPAIR, NDC, 2, 128, PVW).transpose(0, 3, 1, 2, 4)
        ).reshape(NPAIR * 128, NDC * 2 * PVW)

        # zt[d, 6*b_local + m] = z[core_base + b_local, m, d]; bias folded via
        # ones row (zt1 row 64 = 1, wq1 row 64 = bq*scale)
        zt = z[s].reshape(BPC * M, D).T.astype(BF16)
        pk1 = np.zeros((128, 1216), dtype=BF16)
        pk1[:, 0:96] = zt[0:128]
        pk1[0:64, 96:192] = zt[128:192]
        pk1[64, 96:192] = np.float32(1.0)
        pk1[:, 192:704] = wq_s[0:128]
        pk1[0:64, 704:1216] = wq_s[128:192]
        pk1[64, 704:1216] = bq_s

        # zz[12r + 6bb + m] = z[2r + bb, m] + bo
        zz = (z[s] + bo[None, None, :]).reshape(96, D).astype(np.float32)

        in_maps.append({"px": px, "pv": pv, "pk1": pk1, "pk2": pk2, "zz": zz})
    return in_maps


def kernel(**inputs) -> np.ndarray:
    nc = get_nc()
    in_maps = make_in_maps(
        inputs["x"], inputs["z"], inputs["Wq"], inputs["bq"],
        inputs["Wo"], inputs["bo"],
    )
    res = run_bass_kernel_spmd(nc, in_maps, list(range(N_CORES)))
    out = np.concatenate(
        [
            np.asarray(res.results[i]["out"]).reshape(BPC, M, D)
            for i in range(N_CORES)
        ],
        axis=0,
    )
    return out.astype(np.float32)
